# revision 6
# baseline (speedup 1.0000x reference)
"""CRF forward-algorithm loss on 8 Trainium2 NeuronCores — bidirectional.

Math (linear space): Z = r^T (prod_{t=8191..0} D_t expT) w0,  D_t = diag(exp(h[t])),
r = expT[END,:], w0 = onehot(START).  Split at the middle:
  forward  : w_{t+1} = e_t o (expT w_t),      t = 0..4094, then y = expT w_4095
             (implemented as 4096 steps with h-stream [h_0..h_4094, zeros])
  backward : x_k = e o (expT^T x_{k-1}),      h-stream [h_8190, ..., h_4095],
             x_0 = (e_8191 o r)/s0 (host)
  Z * scales = sum_n x_4096[n] * y[n]   (the middle D_4095 cancels)
Each chain runs on a 4-core XOR-closed group (fwd {0..3}, bwd {4..7}) with
tensor parallelism over the output axis: per core a 512-wide slice.
Per step: 32 accumulating matvec matmuls (two N-halves in two PSUM banks; the
A half carries an extra column = colsum for the normalizer S), reciprocal
1/S on DVE (off critical path, overlaps the B half), ACT copies the psum row
halves to SBUF, 4 tiny PE transpose-matmuls scale by 1/S and produce the
[128, 4] column layout, DVE casts to bf16 send tile, GPSIMD broadcasts to the
4-core group via XOR-relative remote DMA, receivers form
w' = gathered * exp(h_t) in one bf16 DVE multiply.

One SPMD program for all 8 cores; direction differences live in the host-side
per-core inputs (mov tiles from expT vs expT^T, h streams, winit).
"""

import sys

if "/opt/trn_rl_repo" not in sys.path:
    sys.path.insert(0, "/opt/trn_rl_repo")

import numpy as np
import ml_dtypes

import concourse.bass as bass
import concourse.bacc as bacc
import concourse.mybir as mybir

START_IDX = 0
END_IDX = 1
K = 2048
SEQH = 8192                  # full sequence
SEQ = SEQH // 2              # per-chain steps
NCORES = 8
GROUP = 4                    # cores per chain
P = 128
SLICE = K // GROUP           # 512 outputs per core
MT = K // P                  # 16 contract chunks of 128
NA = 257                     # A half: 256 outputs + colsum column
NB = 256                     # B half
A_SZ = MT * NA               # 4112
B_SZ = MT * NB               # 4096
MOVW = A_SZ + B_SZ           # 8208
BF16 = mybir.dt.bfloat16
F32 = mybir.dt.float32
NPBF16 = ml_dtypes.bfloat16


def build_probe() -> bass.Bass:
    """Topology probe: each core XOR-broadcasts its logical id to its
    physical Δ=1 and Δ=2 neighbors. Receiver r's outputs are the logical
    ids of phys(r)^1 and phys(r)^2 — enough to reconstruct the two
    XOR-closed 4-core groups ({phys bit2=0} and {bit2=1})."""
    nc = bacc.Bacc(None, target_bir_lowering=False, num_devices=NCORES)
    myid = nc.declare_dram_parameter("myid", [P, 1], F32, isOutput=False)
    nbr = nc.declare_dram_parameter("nbr", [P, 2], F32, isOutput=True)
    myid_sb = nc.alloc_sbuf_tensor("myid_sb", [P, 1], F32)
    nbr_sb = nc.alloc_sbuf_tensor("nbr_sb", [P, 2], F32)
    rsem = nc.alloc_semaphore("rsem")
    lsem = nc.alloc_semaphore("lsem")
    psem = nc.alloc_semaphore("psem")
    dsem = nc.alloc_semaphore("dsem")
    gp, sp = nc.gpsimd, nc.sync
    sp.dma_start(out=myid_sb[:, :], in_=myid[:, :]).then_inc(dsem, 16)
    gp.wait_ge(dsem, 16)
    nc.all_core_barrier()
    for d in (1, 2):
        rd: list = [None] * NCORES
        rd[d] = (0, d)
        gp.remote_dma_broadcast(
            out_ap=nbr_sb[:, d - 1 : d],
            in_ap=myid_sb[:, 0:1],
            remote_sem=rsem,
            local_sem=lsem,
            rdests=rd,
        ).then_inc(psem, 1)
    gp.wait_ge(psem, 2)
    gp.trigger_dma(count=2)
    sp.wait_ge(rsem, 4)
    sp.dma_start(out=nbr[:, :], in_=nbr_sb[:, :]).then_inc(dsem, 16)
    sp.wait_ge(dsem, 32)
    gp.wait_ge(lsem, 32)
    nc.all_core_barrier()
    nc.finalize()
    return nc


_TOPO_GROUPS: "list[list[int]] | None" = None


def discover_groups() -> "list[list[int]]":
    """Run the probe once; return [group_of_core0, other_group]."""
    global _TOPO_GROUPS
    if _TOPO_GROUPS is not None:
        return _TOPO_GROUPS
    from concourse.bass_utils import run_bass_kernel_spmd

    nc = build_probe()
    in_maps = [{"myid": np.full((P, 1), c, np.float32)} for c in range(NCORES)]
    res = run_bass_kernel_spmd(nc, in_maps, core_ids=list(range(NCORES)))
    n1 = [int(res.results[c]["nbr"][0, 0]) for c in range(NCORES)]
    n2 = [int(res.results[c]["nbr"][0, 1]) for c in range(NCORES)]
    for c in range(NCORES):  # sanity: involutions, commuting
        assert n1[n1[c]] == c and n2[n2[c]] == c, (n1, n2)
    g0 = sorted({0, n1[0], n2[0], n2[n1[0]]})
    g1 = sorted(set(range(NCORES)) - set(g0))
    assert len(g0) == 4 and len(g1) == 4, (g0, g1)
    _TOPO_GROUPS = [g0, g1]
    return _TOPO_GROUPS


def build_bass(seq: int, variant: str = "full", warm: int = 0) -> bass.Bass:
    """Device program (SPMD over 8 cores; 4-core XOR comm groups).

    warm > 0 inserts nop-spaced dummy matmuls after the transposes so the PE
    HAM activity monitor never sees a >3.4us idle window during the comm tail
    (else the PE re-throttles to 1.2 GHz every step). The nops+dummies sit
    before the sem_wdone wait and finish well inside the tail latency.
    """
    assert seq % 2 == 0
    comm = variant in ("full", "nowait")
    rwait = variant == "full"
    trans = variant in ("full", "nowait", "nocomm")
    dvework = variant != "mmonly"
    nc = bacc.Bacc(None, target_bir_lowering=False, num_devices=NCORES)

    movq = nc.declare_dram_parameter("movq", [P, MOVW], BF16, isOutput=False)
    hq = nc.declare_dram_parameter("hq", [P, seq * MT], BF16, isOutput=False)
    winit = nc.declare_dram_parameter("winit", [P, MT], BF16, isOutput=False)
    # 4*rank-in-group of this core (slot offset for its broadcast writes)
    srank = nc.declare_dram_parameter("srank", [1, 1], mybir.dt.uint32, isOutput=False)
    wout = nc.declare_dram_parameter("wout", [P, MT], BF16, isOutput=True)
    rec_out = nc.declare_dram_parameter("rec", [1, seq], BF16, isOutput=True)

    movsb = nc.alloc_sbuf_tensor("movsb", [P, MOVW], BF16)
    w_sb = nc.alloc_sbuf_tensor("w_sb", [P, MT], BF16)
    hq_sb = nc.alloc_sbuf_tensor("hq_sb", [P, seq * MT], BF16)
    # 4 sender slots x 4 cols per parity (intra-group broadcasts only)
    graw = nc.alloc_sbuf_tensor("graw", [P, 2 * 4 * GROUP], BF16)
    sendt = nc.alloc_sbuf_tensor("sendt", [P, 8], BF16)        # parity 4+4
    rawrow = nc.alloc_sbuf_tensor("rawrow", [1, SLICE], BF16)  # unscaled matvec row
    rec32_tmp = nc.alloc_sbuf_tensor("rec32_tmp", [1, 1], F32)
    rec_sb = nc.alloc_sbuf_tensor("rec_sb", [1, seq + 1], BF16)  # applied scale; slot t = 1/S_{t-1}, slot0 = 1

    psum_a = nc.alloc_psum_tensor("psum_a", [P, NA], F32)      # row 0 used
    psum_b = nc.alloc_psum_tensor("psum_b", [P, NB], F32)
    psum_t = nc.alloc_psum_tensor("psum_t", [P, 4], F32)       # transposed columns
    psum_dum = nc.alloc_psum_tensor("psum_dum", [P, 512], F32) if warm else None

    # semaphores
    sem_mma = nc.alloc_semaphore("sem_mma")      # A-half matvec done   +1/step
    sem_mmb = nc.alloc_semaphore("sem_mmb")      # B-half matvec done   +1/step
    sem_row = nc.alloc_semaphore("sem_row")      # 1/S ready (DVE)      +1/step
    sem_rawa = nc.alloc_semaphore("sem_rawa")    # row A in SBUF (ACT)  +1/step
    sem_rawb = nc.alloc_semaphore("sem_rawb")    # row B in SBUF (ACT)  +1/step
    sem_tpa = nc.alloc_semaphore("sem_tpa")      # A transposes done    +1/step
    sem_tpb = nc.alloc_semaphore("sem_tpb")      # B transposes done    +1/step
    sem_senda = nc.alloc_semaphore("sem_senda")  # A send tile ready    +1/step
    sem_sendb = nc.alloc_semaphore("sem_sendb")  # B send tile ready    +1/step
    sem_wa = nc.alloc_semaphore("sem_wa")        # w' CG1 cols ready    +1/step
    sem_wb = nc.alloc_semaphore("sem_wb")        # w' CG2 cols ready    +1/step
    rsema = [nc.alloc_semaphore(f"rsema{i}") for i in range(2)]  # +16/same-parity step
    rsemb = [nc.alloc_semaphore(f"rsemb{i}") for i in range(2)]  # +16/same-parity step
    lsem = nc.alloc_semaphore("lsem")            # data send local      +32/step
    psem_d = nc.alloc_semaphore("psem_d")        # data descs written   +2/step
    dma0 = nc.alloc_semaphore("dma0")            # prologue loads

    pe, dve, act, gp, sp = nc.tensor, nc.vector, nc.scalar, nc.gpsimd, nc.sync

    # ---- prologue ----
    dve.memset(rec_sb[0:1, 0:1], 1.0)
    sp.dma_start(out=movsb[:, :], in_=movq[:, :]).then_inc(dma0, 16)
    sp.dma_start(out=w_sb[:, :], in_=winit[:, :]).then_inc(dma0, 16)
    sp.dma_start(out=hq_sb[:, :], in_=hq[:, :]).then_inc(dma0, 16)
    pe.wait_ge(dma0, 48)
    # no remote traffic may be emitted before every core has loaded its state
    nc.all_core_barrier()

    # ---- per-engine monotonic threshold registers ----
    def reg(engine, name, val=0):
        r = engine.alloc_register(name)
        engine.reg_mov(r, val)
        return r

    pe_wa = reg(pe, "pe_wa")
    pe_wb = reg(pe, "pe_wb")
    pe_row = reg(pe, "pe_row")
    pe_rawa = reg(pe, "pe_rawa")
    pe_rawb = reg(pe, "pe_rawb")
    pe_rec = reg(pe, "pe_rec")     # rhs offset into rec_sb
    v_mma = reg(dve, "v_mma")
    v_tpb = reg(dve, "v_tpb")
    a_mma = reg(act, "a_mma")
    a_mmb = reg(act, "a_mmb")
    a_tpa = reg(act, "a_tpa")
    a_ls = reg(act, "a_ls")
    v_rsa = [reg(dve, f"v_rsa{i}") for i in range(2)]
    v_rsb = [reg(dve, f"v_rsb{i}") for i in range(2)]
    v_rec = reg(dve, "v_rec", 1)   # record write offset (lagged: slot t+1)
    v_hq = reg(dve, "v_hq")        # exp(h) tile read offset
    g_senda = reg(gp, "g_senda")
    g_sendb = reg(gp, "g_sendb")
    g_pd = reg(gp, "g_pd")
    GW = 4 * GROUP                 # graw columns per parity (4 slots x 4)
    # broadcast dest offset: graw col GW*par + 4*rank (from the srank input)
    g_off = [gp.alloc_register(f"g_off{i}") for i in range(2)]
    gp.reg_load(g_off[0], srank[0:1, 0:1])
    gp.reg_add(g_off[1], g_off[0], GW)
    v_hq2 = [dve.alloc_register(f"v_hq2_{p}") for p in range(2)]
    dve.reg_mov(v_hq2[0], 0)
    dve.reg_mov(v_hq2[1], 2)

    rdests = [(0, 0), (0, 1), (0, 2), (0, 3), None, None, None, None]

    g_boff = [[gp.alloc_register(f"g_boff{par}{ph}") for ph in range(2)] for par in range(2)]
    for par in range(2):
        for ph in range(2):
            gp.reg_add(g_boff[par][ph], g_off[par], 2 * ph)

    def emit_bcast_prep(par: int, ph: int):
        """Phase ph=0: A output-half (graw cols 4s+{0,1}); ph=1: B half."""
        gp.remote_dma_broadcast(
            out_ap=bass.AP(graw, g_boff[par][ph], [[2 * GW, P], [1, 2]]),
            in_ap=sendt[:, 4 * par + 2 * ph : 4 * par + 2 * ph + 2],
            remote_sem=(rsema if ph == 0 else rsemb)[par],
            local_sem=lsem,
            rdests=rdests,
        ).then_inc(psem_d, 1)

    CG1 = [j2 for j2 in range(MT) if j2 % 4 < 2]   # chunks delivered by A sends
    CG2 = [j2 for j2 in range(MT) if j2 % 4 >= 2]  # chunks delivered by B sends
    # strided free AP over a phase's 8 columns: 4 sender slots x 2 cols
    PH_FREE = [[4, 4], [1, 2]]

    def emit_step(par: int):
        # ---------------- PE ----------------
        # CG1 chunks are gated by the A-phase w' piece, CG2 by the B piece.
        pe.wait_ge(sem_wa, pe_wa)
        pe.reg_add(pe_wa, pe_wa, 1)
        for i, j2 in enumerate(CG1):
            pe.matmul(
                psum_a[0:1, 0:NA],
                w_sb[:, j2 : j2 + 1],
                movsb[:, j2 * NA : (j2 + 1) * NA],
                start=(i == 0),
                stop=False,
            )
        for i, j2 in enumerate(CG1):
            pe.matmul(
                psum_b[0:1, 0:NB],
                w_sb[:, j2 : j2 + 1],
                movsb[:, A_SZ + j2 * NB : A_SZ + (j2 + 1) * NB],
                start=(i == 0),
                stop=False,
            )
        pe.wait_ge(sem_wb, pe_wb)
        pe.reg_add(pe_wb, pe_wb, 1)
        for i, j2 in enumerate(CG2):
            pe.matmul(
                psum_a[0:1, 0:NA],
                w_sb[:, j2 : j2 + 1],
                movsb[:, j2 * NA : (j2 + 1) * NA],
                start=False,
                stop=(i == len(CG2) - 1),
            ).then_maybe_inc((sem_mma, 1) if i == len(CG2) - 1 else None)
        for i, j2 in enumerate(CG2):
            pe.matmul(
                psum_b[0:1, 0:NB],
                w_sb[:, j2 : j2 + 1],
                movsb[:, A_SZ + j2 * NB : A_SZ + (j2 + 1) * NB],
                start=False,
                stop=(i == len(CG2) - 1),
            ).then_maybe_inc((sem_mmb, 1) if i == len(CG2) - 1 else None)
        if trans:
            # transposes apply the 1/S scale for free: out = rawrow_chunk.T @ recip
            pe.wait_ge(sem_row, pe_row)
            pe.reg_add(pe_row, pe_row, 1)
            pe.reg_add(pe_rawa, pe_rawa, 1)
            pe.wait_ge(sem_rawa, pe_rawa)
            rec_pe_ap = bass.AP(rec_sb, pe_rec, [[seq + 1, 1], [1, 1]])
            pe.matmul(psum_t[0:P, 0:1], rawrow[0:1, 0:P], rec_pe_ap, start=True, stop=True)
            pe.matmul(psum_t[0:P, 1:2], rawrow[0:1, P : 2 * P], rec_pe_ap, start=True, stop=True).then_inc(sem_tpa, 1)
            pe.reg_add(pe_rawb, pe_rawb, 1)
            pe.wait_ge(sem_rawb, pe_rawb)
            pe.matmul(psum_t[0:P, 2:3], rawrow[0:1, 2 * P : 3 * P], rec_pe_ap, start=True, stop=True)
            pe.matmul(psum_t[0:P, 3:4], rawrow[0:1, 3 * P : 4 * P], rec_pe_ap, start=True, stop=True).then_inc(sem_tpb, 1)
            pe.reg_add(pe_rec, pe_rec, 1)
        if warm:
            for _ in range(2):
                pe.nop(cycle_cnt=warm)
                pe.matmul(
                    psum_dum[0:1, 0:512], w_sb[:, 0:1], movsb[:, 0:512],
                    start=True, stop=True,
                )

        # ---------------- ACT: the whole A-phase tail (rawA then copyA) ----
        if trans:
            act.reg_add(a_mma, a_mma, 1)
            act.wait_ge(sem_mma, a_mma)
            act.activation(
                rawrow[0:1, 0:NB], psum_a[0:1, 0:NB],
                mybir.ActivationFunctionType.Copy,
            ).then_inc(sem_rawa, 1)
            act.reg_add(a_tpa, a_tpa, 1)
            act.wait_ge(sem_tpa, a_tpa)
            if comm:
                act.wait_ge(lsem, a_ls)  # my sends through t-1 left sendt
                act.reg_add(a_ls, a_ls, 32)
            act.activation(
                sendt[:, 4 * par : 4 * par + 2], psum_t[0:P, 0:2],
                mybir.ActivationFunctionType.Copy,
            ).then_inc(sem_senda, 1)
            act.reg_add(a_mmb, a_mmb, 1)
            act.wait_ge(sem_mmb, a_mmb)
            act.activation(
                rawrow[0:1, NB:SLICE], psum_b[0:1, 0:NB],
                mybir.ActivationFunctionType.Copy,
            ).then_inc(sem_rawb, 1)

        # ---------------- DVE: recip + copyB + the w' multiplies ----
        dve.reg_add(v_mma, v_mma, 1)
        dve.wait_ge(sem_mma, v_mma)
        if dvework:
            rec_ap = bass.AP(rec_sb, v_rec, [[seq + 1, 1], [1, 1]])
            dve.reciprocal(rec32_tmp[0:1, 0:1], psum_a[0:1, NB:NA])
            dve.drain()
            dve.tensor_copy(rec_ap, rec32_tmp[0:1, 0:1]).then_inc(sem_row, 1)
            dve.reg_add(v_rec, v_rec, 1)
        if trans:
            dve.reg_add(v_tpb, v_tpb, 1)
            dve.wait_ge(sem_tpb, v_tpb)
            if comm:
                dve.tensor_copy(sendt[:, 4 * par + 2 : 4 * par + 4], psum_t[0:P, 2:4]).then_inc(sem_sendb, 1)
            else:
                # own slice locally (normally delivered by the self-dest broadcast)
                dve.tensor_copy(
                    bass.AP(graw, GW * par, [[2 * GW, P], [1, 4]]),
                    psum_t[0:P, 0:4],
                ).then_inc(sem_sendb, 1)
                dve.drain()
        # w' piece for CG1 (cols 4s+{0,1}) then CG2 (cols 4s+{2,3})
        for ph, (vr, rs, sem_w) in enumerate(
            ((v_rsa, rsema, sem_wa), (v_rsb, rsemb, sem_wb))
        ):
            if rwait:
                dve.reg_add(vr[par], vr[par], 8)
                dve.wait_ge(rs[par], vr[par])
            dve.tensor_tensor(
                bass.AP(w_sb, 2 * ph, [[MT, P]] + PH_FREE),
                bass.AP(graw, GW * par + 2 * ph, [[2 * GW, P]] + PH_FREE),
                bass.AP(hq_sb, v_hq2[ph], [[seq * MT, P]] + PH_FREE),
                op=mybir.AluOpType.mult,
            ).then_inc(sem_w, 1)
        dve.reg_add(v_hq2[0], v_hq2[0], MT)
        dve.reg_add(v_hq2[1], v_hq2[1], MT)

        # ---------------- GPSIMD ----------------
        # Preps at the top (desc-gen overlaps the matvec); trigger each phase
        # once its send tile is ready. Flow control via the rsem chains.
        if comm:
            emit_bcast_prep(par, 0)
            emit_bcast_prep(par, 1)
            gp.reg_add(g_senda, g_senda, 1)
            gp.wait_ge(sem_senda, g_senda)
            gp.reg_add(g_pd, g_pd, 1)
            gp.wait_ge(psem_d, g_pd)
            gp.trigger_dma(count=1)
            gp.reg_add(g_sendb, g_sendb, 1)
            gp.wait_ge(sem_sendb, g_sendb)
            gp.reg_add(g_pd, g_pd, 1)
            gp.wait_ge(psem_d, g_pd)
            gp.trigger_dma(count=1)

    # ---- main loop: 2-step parity unroll ----
    with nc.Fori(0, seq // 2):
        emit_step(0)
        emit_step(1)

    # ---- epilogue ----
    if dvework:
        sp.wait_ge(sem_row, seq)
        sp.dma_start(out=rec_out[:, :], in_=rec_sb[:, 0:seq]).then_inc(dma0, 16)
    sp.wait_ge(sem_wb, seq)
    sp.dma_start(out=wout[:, :], in_=w_sb[:, :]).then_inc(dma0, 16)
    sp.wait_ge(dma0, 80 if dvework else 64)
    if comm:
        gp.wait_ge(lsem, 32 * seq)
    nc.all_core_barrier()
    nc.finalize()
    return nc


def _mov_tile(M: np.ndarray, colsum: np.ndarray) -> np.ndarray:
    """Pack one core's moving tile.

    M: [512 out, 2048 contract] bf16 — out index col, contract index j.
    Column order per contract chunk j2 (A region then B region):
      A: cols 0..255 then the colsum column;  B: cols 256..511.
    mov[q, A_region j2*257 + c] = M[c, 128 j2 + q]
    """
    Mr = np.ascontiguousarray(M.reshape(SLICE, MT, P))          # [col, j2, q]
    A = Mr[:NB].transpose(2, 1, 0)                              # [q, j2, 256]
    S = colsum.reshape(MT, P).T[:, :, None]                     # [q, j2, 1]
    Areg = np.concatenate([A, S], axis=2).reshape(P, A_SZ)      # [q, 16*257]
    Breg = np.ascontiguousarray(Mr[NB:].transpose(2, 1, 0)).reshape(P, B_SZ)
    return np.ascontiguousarray(np.concatenate([Areg, Breg], axis=1))


def prep_inputs(h: np.ndarray, transitions: np.ndarray, seq: int, groups=None):
    """Host-side layout of per-core inputs (seq = per-chain steps).

    groups[0] (containing core 0) runs the forward chain, groups[1] the
    backward chain; each core's slice index is its rank within its group.
    """
    if groups is None:
        groups = [[0, 1, 2, 3], [4, 5, 6, 7]]
    h32 = h.astype(np.float32)
    T32 = transitions.astype(np.float32)
    expT32 = np.exp(T32)
    expTq = expT32.astype(NPBF16)
    expTq32 = expTq.astype(np.float32)
    colsum_f = expTq32.sum(axis=0).astype(NPBF16)   # fwd: sum over next
    rowsum_f = expTq32.sum(axis=1).astype(NPBF16)   # bwd: sum over prev

    # h streams (length seq each); layout hq[q, t*16 + j2] = e[t, 128 j2 + q]
    def hq_pack(stream32: np.ndarray) -> np.ndarray:
        e = np.exp(stream32).astype(NPBF16)
        return np.ascontiguousarray(
            e.reshape(seq, MT, P).transpose(2, 0, 1).reshape(P, seq * MT)
        )

    sf = np.concatenate([h32[: seq - 1], np.zeros((1, K), np.float32)], axis=0)
    sb = np.ascontiguousarray(h32[seq - 1 : 2 * seq - 1][::-1])  # h[8190]..h[4095]
    hq_f = hq_pack(sf)
    hq_b = hq_pack(sb)

    # winit: fwd = onehot(START); bwd = (e_8191 * expT[END,:]) / s0
    wi_f = np.zeros((P, MT), dtype=NPBF16)
    wi_f[START_IDX % P, START_IDX // P] = 1.0
    x0 = np.exp(h32[2 * seq - 1].astype(np.float64)) * np.exp(
        T32[END_IDX].astype(np.float64)
    )
    s0 = float(x0.sum())
    wi_b = (x0 / s0).astype(NPBF16).reshape(MT, P).T.copy()

    in_maps = []
    for r in range(NCORES):
        gi = 0 if r in groups[0] else 1
        g = groups[gi].index(r)
        meta = {"srank": np.array([[4 * g]], np.uint32)}
        if gi == 0:    # forward: out = next, contract = prev
            M = expTq[SLICE * g : SLICE * (g + 1), :]           # [512 next, 2048 prev]
            mov = _mov_tile(M, colsum_f)
            in_maps.append({"movq": mov, "hq": hq_f, "winit": wi_f, **meta})
        else:          # backward: out = prev, contract = next -> moving = expT^T slice
            M = np.ascontiguousarray(expTq[:, SLICE * g : SLICE * (g + 1)].T)
            mov = _mov_tile(M, rowsum_f)
            in_maps.append({"movq": mov, "hq": hq_b, "winit": wi_b, **meta})
    return in_maps, s0


def finalize(results, s0: float, seq: int, groups=None):
    """Combine device outputs into the scalar answer (host, fp64)."""
    if groups is None:
        groups = [[0, 1, 2, 3], [4, 5, 6, 7]]

    def unpack_w(res):
        return res["wout"].astype(np.float64).T.reshape(-1)     # w[c*128+q]

    y = unpack_w(results[groups[0][0]])                         # fwd chain
    x = unpack_w(results[groups[1][0]])                         # bwd chain
    rec_f = results[groups[0][0]]["rec"].reshape(-1).astype(np.float64)
    rec_b = results[groups[1][0]]["rec"].reshape(-1).astype(np.float64)
    dot = float(np.dot(x, y))
    ans = (
        np.log(dot)
        - np.sum(np.log(rec_f))
        - np.sum(np.log(rec_b))
        + np.log(s0)
    )
    return np.float32(ans)


def kernel(h: np.ndarray, transitions: np.ndarray) -> np.ndarray:
    from concourse.bass_utils import run_bass_kernel_spmd

    try:
        groups = discover_groups()
    except Exception:
        # Probe hiccup: fall back to the grouping verified on this host.
        groups = [[0, 1, 2, 3], [4, 5, 6, 7]]
    nc = build_bass(SEQ)
    in_maps, s0 = prep_inputs(np.asarray(h), np.asarray(transitions), SEQ, groups)
    res = run_bass_kernel_spmd(nc, in_maps, core_ids=list(range(NCORES)))
    return finalize(res.results, s0, SEQ, groups)


if __name__ == "__main__":
    import reference

    inputs = {k: np.asarray(v) for k, v in reference.setup_inputs().items()}
    out = kernel(**inputs)
    print("kernel:", out)
